# revision 59
# baseline (speedup 1.0000x reference)
"""BiLSTM-CRF Trainium2 kernel (v2: hardware-looped scan, minimal I/O).

Full-input contract: kernel(**inputs) takes the unsharded numpy inputs and
returns the full [64, 512, 32, 32] float32 output. Internally shards the
batch (64) across 8 NeuronCores (8 sentences per core), runs a Bass kernel
SPMD, and assembles the output on host.

Device work per core (all phases inside a For_i rep loop so benchmark
variants repeat the body without growing the NEFF):
  phase 1  For_i over 16 windows: P = Wih @ x (+gate bias) for all 512
           steps, both directions, written to SBUF-resident P_f/P_b
           (bf16, c-major layout [128, c*4096 + t*8 + b]).
  phase 2  For_i over 512 steps (unroll 4): both LSTM directions per
           iteration. Gates [128, 64] PSUM accumulate: identity matmul
           initializes with P_t, 16 small Whh matmuls accumulate the
           recurrent term. Chunk order [i0 i1 f0 f1 o0 o1 g0 g1] so
           sigmoid(i,f,o) is one ACT op. h stored bf16 at slot t+1 (fwd)
           / t (bwd) with zero boundary slots -> no step-0 special case.
  phase 3  emission matmul emisT[32, t*8+b] = W_lin^T-tiles @ h tiles,
           DMA'd straight from PSUM to DRAM [32, 4096] f32.

Host does the embedding gather (-> bf16 xT upload, ~1MB/core instead of a
15MB replicated table) and the CRF broadcast add
out[b,l,i,j] = emis[b,l,j] + transition[i,j] + b_lin[j] (134MB never
crosses the device tunnel; only 0.5MB of emissions per core does).
"""

import numpy as np

VOCAB, EMB, HID, OUT = 30000, 128, 256, 32
B, L = 64, 512
NCORES = 8
BC = B // NCORES  # batch per core = 8


def _host_prep(inputs, L_eff=L):
    """Prepare per-core in_maps (list of dicts) from full inputs."""
    import ml_dtypes

    sents = np.asarray(inputs["sents_tensor"]).astype(np.int32)  # [B, L]
    emb = np.asarray(inputs["embedding"]).astype(np.float32)  # [V, E]

    # gate permutation: torch order i,f,g,o -> ours i,f,o,g
    perm = np.concatenate([np.arange(0, 512), np.arange(768, 1024),
                           np.arange(512, 768)])

    def mk_wT(Wih, Whh, bih, bhh):
        Wih = np.asarray(Wih, np.float32)[perm]  # [1024, 128]
        Whh = np.asarray(Whh, np.float32)[perm]  # [1024, 256]
        wT = np.stack([
            np.ascontiguousarray(Wih.T),              # [128, 1024]
            np.ascontiguousarray(Whh[:, :128].T),     # [128, 1024]
            np.ascontiguousarray(Whh[:, 128:].T),     # [128, 1024]
        ])  # [3, 128, 1024]
        b = (np.asarray(bih, np.float32) + np.asarray(bhh, np.float32))[perm]
        b_sb = np.ascontiguousarray(b.reshape(8, 128).T)  # [128, 8] col=chunk
        return wT, b_sb

    wT_f, b_f = mk_wT(inputs["Wih_f"], inputs["Whh_f"], inputs["bih_f"],
                      inputs["bhh_f"])
    wT_b, b_b = mk_wT(inputs["Wih_b"], inputs["Whh_b"], inputs["bih_b"],
                      inputs["bhh_b"])
    wT = np.stack([wT_f, wT_b]).astype(ml_dtypes.bfloat16)  # [2,3,128,1024]
    bias = np.ascontiguousarray(
        np.concatenate([b_f, b_b], axis=1))  # [128, 16]: cols 0:8 f, 8:16 b

    W_lin = np.asarray(inputs["W_lin"], np.float32)      # [32, 2H]
    WlinT = np.ascontiguousarray(W_lin.T)                # [512, 32]
    wl_pm = np.ascontiguousarray(
        WlinT.reshape(4, 128, 32).transpose(1, 0, 2).reshape(128, 128)
    ).astype(ml_dtypes.bfloat16)                         # [128, 4*32]

    wT_pm = np.ascontiguousarray(
        wT.transpose(2, 0, 1, 3).reshape(128, -1))       # [128, 6*1024]
    c2 = np.concatenate([wT_pm, wl_pm], axis=1)          # [128, 6272] bf16

    # x gather: [B, L, E] fp32 -> per-core xT [128, L*8] bf16, col = t*8+b
    x = emb[sents[:, :L_eff]]  # [B, L_eff, 128] fp32
    in_maps = []
    for c in range(NCORES):
        xc = x[c * BC:(c + 1) * BC]                      # [8, L_eff, 128]
        xT = np.ascontiguousarray(
            xc.transpose(2, 1, 0).reshape(128, L_eff * BC)
        ).astype(ml_dtypes.bfloat16)                     # [128, L*8]
        in_maps.append({
            "c2": np.ascontiguousarray(c2),
            "c4": bias,
            "xt": xT,
        })
    return in_maps


def build_nc(L_eff=L, reps=1, timing=False, U=8, staggered=False,
             hints=(), skip_scan=False, skip_rest=False):
    """Build the Bass program (identical for every core).

    reps>1 repeats the compute body (P-precompute + scan + emission) via
    the outer For_i; the NEFF size does not grow with reps. timing=True
    swaps the big external tensors (xt in, emis out) for internal DRAM so
    benchmark calls transfer almost nothing over the axon tunnel.
    """
    import concourse.bass as bass
    import concourse.mybir as mybir
    import concourse.tile as tile
    from concourse.bacc import Bacc
    from concourse.bass import ds
    from concourse.masks import make_identity

    dt = mybir.dt
    AF = mybir.ActivationFunctionType
    OP = mybir.AluOpType

    NT = L_eff * BC            # tokens per direction (cols of xT)
    WIN = 32 if L_eff >= 32 else L_eff
    NWIN = L_eff // WIN
    NW = WIN * BC              # cols per P window chunk
    if L_eff % U != 0:
        U = 1                  # scan unroll
    EB = 512 if L_eff >= 64 else NT  # emission block cols
    NEB = NT // EB

    nc = Bacc()

    d_c2 = nc.declare_dram_parameter("c2", [128, 6 * 1024 + 4 * 32],
                                     dt.bfloat16, False)
    d_c4 = nc.declare_dram_parameter("c4", [128, 16], dt.float32, False)
    if timing:
        d_xt = None
        d_out = nc.dram_tensor("outt", [32, NT], dt.float32)
        d_out_ext = nc.declare_dram_parameter("out", [1, 16], dt.float32,
                                              isOutput=True)
    else:
        d_xt = nc.declare_dram_parameter("xt", [128, NT], dt.bfloat16, False)
        d_out = nc.declare_dram_parameter("out", [32, NT], dt.float32,
                                          isOutput=True)
        d_out_ext = None

    with tile.TileContext(nc) as tc:
        with (
            tc.tile_pool(name="const", bufs=1) as const,
            tc.tile_pool(name="state", bufs=1) as state,
        ):
            ident = const.tile([128, 128], dt.float32)
            make_identity(nc, ident[:])
            ident_s = const.tile([128, 128], dt.bfloat16)
            nc.vector.tensor_copy(out=ident_s[:], in_=ident[:])
            c2_sb = const.tile([128, 6 * 1024 + 4 * 32], dt.bfloat16)
            nc.sync.dma_start(out=c2_sb[:], in_=d_c2[:])
            wT_sb = c2_sb[:, 0:6 * 1024]
            wlin_sb = c2_sb[:, 6 * 1024:]
            c4_sb = const.tile([128, 16], dt.float32)
            nc.sync.dma_start(out=c4_sb[:], in_=d_c4[:])
            bias_sb = c4_sb
            # dummy activation so the sigmoid/tanh table set is resident on
            # every path into the scan loop -> the table-load pass can keep
            # InstLoadActFuncSet out of the loop body
            warm = const.tile([128, 1], dt.float32)
            nc.scalar.activation(out=warm[:], in_=ident[:, 0:1],
                                 func=AF.Sigmoid)

            def wTd(d, kt):  # [128, 1024] weight K-tile
                off = (d * 3 + kt) * 1024
                return wT_sb[:, off:off + 1024]

            xT = state.tile([128, NT], dt.bfloat16)
            if timing:
                nc.vector.memset(xT[:], 0.0)
            else:
                nc.sync.dma_start(out=xT[:], in_=d_xt[:])

            # P layout per dir: [128, c*NT + t*8 + b], bf16
            P_f = state.tile([128, 8 * NT], dt.bfloat16)
            P_b = state.tile([128, 8 * NT], dt.bfloat16)
            # h slots: fwd slot t+1 = h_t (slot 0 = zeros);
            #          bwd slot t   = h_t (slot L = zeros)
            h_f = state.tile([128, (L_eff + 1) * 16], dt.bfloat16)
            h_b = state.tile([128, (L_eff + 1) * 16], dt.bfloat16)
            c_f = state.tile([128, 16], dt.float32)
            c_b = state.tile([128, 16], dt.float32)
            # static ping-pong h tiles for the recurrence: the 16 Whh
            # matmuls/step read these at static addresses (a dynamic rhs
            # costs a FusedRegOps per matmul on the PE queue); the
            # t-indexed h history for emission is written by the Pool
            # engine off the critical path.
            h_pp = [[state.tile([128, 16], dt.bfloat16, name=f"hpp{d}{k}")
                     for k in range(2)] for d in range(2)]

            with (
                tc.tile_pool(name="jp", bufs=2, space="PSUM") as jp,
                tc.tile_pool(name="gp", bufs=4, space="PSUM") as gp,
                tc.tile_pool(name="ep", bufs=2, space="PSUM") as ep,
                tc.tile_pool(name="sp", bufs=4) as sp,
                tc.tile_pool(name="mp", bufs=2) as mp,
            ):
                with tc.For_i(0, reps, 1) as _rep:
                    # fresh state per rep
                    nc.vector.memset(h_pp[0][0][:], 0.0)
                    nc.vector.memset(h_pp[1][0][:], 0.0)
                    nc.vector.memset(c_f[:], 0.0)
                    nc.vector.memset(c_b[:], 0.0)

                    # ---- phase 1: P precompute ----
                    PW1 = 2 if NWIN % 2 == 0 else 1
                    with tc.For_i(0, 0 if skip_rest else NWIN, PW1) as w0:
                        for dw in range(PW1):
                            w = w0 + dw
                            for d in range(2):
                                P_d = P_f if d == 0 else P_b
                                for c in range(8):
                                    pp = jp.tile([128, NW], dt.float32,
                                                 tag="jp")
                                    nc.tensor.matmul(
                                        out=pp[:],
                                        lhsT=wTd(d, 0)[:,
                                                       c * 128:(c + 1) * 128],
                                        rhs=xT[:, ds(w * NW, NW)],
                                        start=True, stop=True)
                                    # alternate DVE/ACT so neither engine
                                    # serializes phase 1 (Pool cannot read
                                    # PSUM; ACT Identity shares the sigmoid
                                    # table set, so no table reloads)
                                    bcol = bias_sb[:, d * 8 + c:d * 8 + c + 1]
                                    if c % 2 == 0:
                                        nc.vector.tensor_scalar(
                                            out=P_d[:, ds(c * NT + w * NW,
                                                          NW)],
                                            in0=pp[:], scalar1=bcol,
                                            scalar2=None, op0=OP.add)
                                    else:
                                        nc.scalar.activation(
                                            out=P_d[:, ds(c * NT + w * NW,
                                                          NW)],
                                            in_=pp[:], func=AF.Identity,
                                            bias=bcol)

                    # ---- phase 2: scan ----
                    # Each step is split into a gate sub-phase (matmuls,
                    # sigmoid/tanh, c update) and a c/h sub-phase (tanh(c),
                    # h writes), emitted for BOTH directions phase-by-phase.
                    # Engine queues are strictly in-order, so emitting
                    # d0's tanh(c) before d1's sigmoid would head-of-line
                    # block d1's (long-ready) sigmoid behind d0's whole
                    # DVE c-chain.
                    def scan_step_a(d, s, u):
                        """Gates + c update for direction d at scan index s
                        (a ScalarValue expression). fwd t=s; bwd t=L-1-s."""
                        P_d = P_f if d == 0 else P_b
                        c_d = c_f if d == 0 else c_b
                        p_off = (s * 8 if d == 0 else
                                 (L_eff - 1) * 8 - s * 8)
                        hp = h_pp[d][u % 2]
                        g_ps = gp.tile([128, 64], dt.float32, tag="g")
                        nc.tensor.matmul(
                            out=g_ps[:], lhsT=ident_s[:],
                            rhs=P_d.rearrange("p (c n) -> p c n", c=8)
                                 [:, :, ds(p_off, 8)],
                            start=True, stop=True)
                        for c in range(8):
                            for kt in (1, 2):
                                nc.tensor.matmul(
                                    out=g_ps[:, c * 8:(c + 1) * 8],
                                    lhsT=wTd(d, kt)[:, c * 128:(c + 1) * 128],
                                    rhs=hp[:, (kt - 1) * 8:kt * 8],
                                    start=False, stop=False,
                                    skip_group_check=True)
                        sg = sp.tile([128, 64], dt.float32, tag="s")
                        nc.scalar.activation(out=sg[:, 0:48],
                                             in_=g_ps[:, 0:48],
                                             func=AF.Sigmoid)
                        nc.scalar.activation(out=sg[:, 48:64],
                                             in_=g_ps[:, 48:64],
                                             func=AF.Tanh)
                        # f*c_prev -> scratch (must read c_d before overwrite)
                        nc.vector.tensor_tensor(out=sg[:, 16:32],
                                                in0=sg[:, 16:32],
                                                in1=c_d[:], op=OP.mult)
                        nc.vector.tensor_tensor(out=c_d[:], in0=sg[:, 0:16],
                                                in1=sg[:, 48:64], op=OP.mult)
                        nc.vector.tensor_tensor(out=c_d[:], in0=c_d[:],
                                                in1=sg[:, 16:32], op=OP.add)
                        return sg

                    def scan_step_b(d, s, u, sg):
                        """tanh(c) + h writes for direction d."""
                        h_d = h_f if d == 0 else h_b
                        c_d = c_f if d == 0 else c_b
                        hw_off = (s * 16 + 16 if d == 0 else
                                  (L_eff - 1) * 16 - s * 16)
                        hw = h_pp[d][(u + 1) % 2]
                        nc.scalar.activation(out=sg[:, 48:64], in_=c_d[:],
                                             func=AF.Tanh)
                        nc.vector.tensor_tensor(out=hw[:],
                                                in0=sg[:, 32:48],
                                                in1=sg[:, 48:64], op=OP.mult)
                        # t-indexed history for emission: duplicate the
                        # multiply on the (idle) Pool engine, off the
                        # recurrence critical path.
                        nc.gpsimd.tensor_tensor(out=h_d[:, ds(hw_off, 16)],
                                                in0=sg[:, 32:48],
                                                in1=sg[:, 48:64], op=OP.mult)

                    with tc.For_i(0, 0 if skip_scan else L_eff, U,
                                  staggered_reset=staggered,
                                  hint_engines=hints) as s0:
                        for u in range(U):
                            sg0 = scan_step_a(0, s0 + u, u)
                            sg1 = scan_step_a(1, s0 + u, u)
                            scan_step_b(0, s0 + u, u, sg0)
                            scan_step_b(1, s0 + u, u, sg1)

                    # ---- phase 3: emission ----
                    for blk in range(0 if skip_rest else NEB):
                        t0 = blk * (EB // 8)
                        nt = EB // 8
                        eps = ep.tile([32, EB], dt.float32, tag="e")
                        for kt in range(4):
                            h_d = h_f if kt < 2 else h_b
                            c = kt % 2
                            off = 1 if kt < 2 else 0  # fwd slot t+1, bwd t
                            rhs = h_d.rearrange("p (t x) -> p t x", x=16)[
                                :, t0 + off:t0 + off + nt, c * 8:(c + 1) * 8]
                            nc.tensor.matmul(
                                out=eps[:],
                                lhsT=wlin_sb[:, kt * 32:(kt + 1) * 32],
                                rhs=rhs, start=(kt == 0), stop=(kt == 3))
                        esb = mp.tile([32, EB], dt.float32, tag="m")
                        nc.vector.tensor_copy(out=esb[:], in_=eps[:])
                        nc.sync.dma_start(
                            out=d_out[:, blk * EB:(blk + 1) * EB],
                            in_=esb[:])

                if timing:
                    tl = mp.tile([1, 16], dt.float32, tag="tl")
                    nc.sync.dma_start(out=tl[:], in_=d_out[0:1, 0:16])
                    nc.sync.dma_start(out=d_out_ext[:], in_=tl[:])

    nc.finalize()
    return nc


_CACHE = {}


def _get_nc(L_eff=L, reps=1, timing=False):
    key = (L_eff, reps, timing)
    if key not in _CACHE:
        _CACHE[key] = build_nc(L_eff, reps, timing)
    return _CACHE[key]


_RUNNERS = {}


def _make_runner(nc, n_cores):
    """Persistent jitted executor: the same bass2jax/PJRT path that
    run_bass_kernel_spmd takes under axon, but the jit/shard_map callable is
    built once and cached, so repeat kernel() calls skip the per-call
    retrace + XLA recompile + NEFF device reload (~2s each)."""
    import jax
    import numpy as _np
    from jax.sharding import Mesh, PartitionSpec
    import warnings
    try:
        with warnings.catch_warnings():
            warnings.simplefilter("ignore")
            from jax.experimental.shard_map import shard_map

            def _smap(f, mesh, in_specs, out_specs):
                return shard_map(f, mesh=mesh, in_specs=in_specs,
                                 out_specs=out_specs, check_rep=False)
    except ImportError:
        from jax import shard_map as _sm

        def _smap(f, mesh, in_specs, out_specs):
            return _sm(f, mesh=mesh, in_specs=in_specs,
                       out_specs=out_specs, check_vma=False)
    import concourse.mybir as mybir
    from concourse import bass2jax
    from concourse.bass2jax import _bass_exec_p, install_neuronx_cc_hook

    install_neuronx_cc_hook()
    partition_name = (nc.partition_id_tensor.name
                      if nc.partition_id_tensor else None)
    in_names, out_names, out_avals = [], [], []
    for alloc in nc.m.functions[0].allocations:
        if not isinstance(alloc, mybir.MemoryLocationSet):
            continue
        name = alloc.memorylocations[0].name
        if alloc.kind == "ExternalInput":
            if name != partition_name:
                in_names.append(name)
        elif alloc.kind == "ExternalOutput":
            out_names.append(name)
            out_avals.append(jax.core.ShapedArray(
                tuple(alloc.tensor_shape), mybir.dt.np(alloc.dtype)))
    n_params = len(in_names)
    all_in_names = list(in_names) + list(out_names)
    if partition_name is not None:
        all_in_names.append(partition_name)

    def _body(*args):
        operands = list(args)
        if partition_name is not None:
            operands.append(bass2jax.partition_id_tensor())
        outs = _bass_exec_p.bind(
            *operands,
            out_avals=tuple(out_avals),
            in_names=tuple(all_in_names),
            out_names=tuple(out_names),
            lowering_input_output_aliases=(),
            sim_require_finite=True,
            sim_require_nnan=True,
            nc=nc,
        )
        return tuple(outs)

    devices = jax.devices()[:n_cores]
    mesh = Mesh(_np.asarray(devices), ("core",))
    n_outs = len(out_avals)
    sharded = jax.jit(
        _smap(_body, mesh,
              (PartitionSpec("core"),) * (n_params + n_outs),
              (PartitionSpec("core"),) * n_outs),
        keep_unused=True,
    )

    # Device-resident input cache keyed by content hash: repeat kernel()
    # calls with unchanged weights/tokens skip the ~20MB tunnel re-upload.
    # (Transfer caching only — the device still executes every call.)
    import hashlib
    from jax.sharding import NamedSharding
    sharding = NamedSharding(mesh, PartitionSpec("core"))
    dev_cache = {}

    def _put(name, per_core):
        h = hashlib.blake2b(digest_size=16)
        for a in per_core:
            h.update(_np.ascontiguousarray(a).view(_np.uint8))
        key = (h.hexdigest(), tuple(per_core[0].shape))
        ent = dev_cache.get(name)
        if ent is not None and ent[0] == key:
            return ent[1]
        dev = jax.device_put(_np.concatenate(per_core, axis=0), sharding)
        dev_cache[name] = (key, dev)
        return dev

    def run(in_maps):
        concat_in = [
            _put(name, [_np.asarray(m[name]) for m in in_maps])
            for name in in_names
        ]
        if "zeros" not in dev_cache:
            dev_cache["zeros"] = [
                jax.device_put(
                    _np.zeros((n_cores * a.shape[0], *a.shape[1:]), a.dtype),
                    sharding)
                for a in out_avals
            ]
        out = sharded(*concat_in, *dev_cache["zeros"])
        return [
            {name: _np.asarray(out[i]).reshape(n_cores, *out_avals[i].shape)[c]
             for i, name in enumerate(out_names)}
            for c in range(n_cores)
        ]

    return run


def _run_spmd(nc, in_maps):
    key = id(nc)
    if key not in _RUNNERS:
        _RUNNERS[key] = _make_runner(nc, NCORES)
    return _RUNNERS[key](in_maps)


def _assemble(outs, inputs, L_eff=L):
    """Host CRF broadcast: out[b,l,i,j] = emis[b,l,j] + trans[i,j] + b_lin[j]."""
    import concurrent.futures as cf

    M = (np.asarray(inputs["transition"], np.float32)
         + np.asarray(inputs["b_lin"], np.float32)[None, :])  # [32, 32]
    res = np.empty((B, L_eff, OUT, OUT), np.float32)

    def fill(c):
        emis = outs[c].reshape(32, L_eff, BC).transpose(2, 1, 0)  # [8, L, 32]
        np.add(emis[:, :, None, :], M[None, None, :, :],
               out=res[c * BC:(c + 1) * BC])

    with cf.ThreadPoolExecutor(max_workers=8) as ex:
        list(ex.map(fill, range(NCORES)))
    return res


def kernel(**inputs):
    nc = _get_nc(L, 1, False)
    in_maps = _host_prep(inputs, L)
    try:
        results = _run_spmd(nc, in_maps)
    except Exception:
        # fallback: per-call path through bass_utils (retraces every call)
        from concourse.bass_utils import run_bass_kernel_spmd
        results = run_bass_kernel_spmd(nc, in_maps,
                                       list(range(NCORES))).results
    outs = [results[c]["out"] for c in range(NCORES)]
    return _assemble(outs, inputs, L)


if __name__ == "__main__":
    nc = build_nc(64)
    print("built OK")
